# revision 4
# baseline (speedup 1.0000x reference)
"""CopyGenerator kernel for 8 trn2 NeuronCores (vocab-tensor-parallel, fp8).

Math (per reference):
    cp      = sigmoid(hidden @ w_copy + b_copy)            # copy gate, per token
    logits  = hidden @ W_gen.T + b_gen                     # [tok, V]
    prob    = softmax(logits)
    attn    = softmax(mask(hidden @ context.T per batch))  # [tok, S]
    p_g     = prob*(1-cp); p_g[t,b,src[b,s]] += attn*cp
    out     = log(p_g) + C

Sharding: vocab split 8 ways (4000/core + 32 pad/dup columns, fp8 W).
Cross-core softmax denominator via 4 pipelined tiny AllReduces. The big
matmul runs in fp8e4 DoubleRow mode (2 k-planes per matmul). Logits (not
exponentials) are kept in SBUF, so the final pass is a per-token bias add
(gpsimd) instead of a full Ln pass; only the per-batch 64-column scatter
blocks go through exp/add/ln. The scatter-add is SPMD-uniform via a
host-side permutation: batch b's owned vocab lands in a 64-col block at
blk0(b) = (b//7)*504 + (b%7)*64 (never straddles a 504-wide v-tile).
Duplicate (b,s)->same-vocab columns carry W=0 and are merged on the host
in prob space using a designated all-pad column as the baseline; every
core has exactly 32 zero-weight columns whose exp(0)*(1-cp) contribution
to z is subtracted analytically, so z is exact.

Token layout is batch-outer: n = b*64 + t.
"""

import sys
import time

sys.path.insert(0, "/opt/trn_rl_repo")

import numpy as np

import concourse.bass as bass
import concourse.mybir as mybir
import concourse.tile as tile
from bass_rust import SyncInfo
from concourse.bass_utils import run_bass_kernel_spmd

FP32 = mybir.dt.float32
BF16 = mybir.dt.bfloat16
FP8 = mybir.dt.float8e4
FP16 = mybir.dt.float16
AF = mybir.ActivationFunctionType
OP = mybir.AluOpType
DR = mybir.MatmulPerfMode.DoubleRow

NCORE = 8
T, B, S, H, V = 64, 32, 64, 1024, 32000
NTOK = T * B              # 2048
KT = H // 128             # 8 k-tiles
KP = KT // 2              # 4 fp8 k-pairs
VS = V // NCORE           # 4000 vocab / core
WCOLS = 4032              # 4000 + 32 dup/pad columns (8*504)
VN = WCOLS // 8           # 504 per v-tile
TT_N = NTOK // 128        # 16 token tiles
NG = 4                    # z-allreduce groups
GT = TT_N // NG           # token tiles per group
CW = S + 1                # ctx cols per batch incl. copy-gate column
C_CONST = 0.1712209
NEG_BIG = -60000.0  # fits fp16
HS = 16.0                 # hidden fp8 scale
WS = 1024.0               # W fp8 scale
INV = 1.0 / (HS * WS)
NZERO = float(WCOLS - VS)  # zero-weight cols per core (exactly 32)


def blk0(b):
    """Scatter block base column for batch b (within-[0,504) aligned)."""
    return (b // 7) * VN + (b % 7) * 64


def _split_multi_waits(nc):
    """This container's walrus accepts at most 1 sem-wait per instruction
    (2 on EventSemaphore). Tile's exit drain exceeds that; hoist extras onto
    EventSemaphore carriers inserted right before the offender."""
    for f in nc.m.functions:
        for b in f.blocks:
            out, changed = [], False
            for inst in list(b.instructions):
                si = inst.sync_info
                if si is not None:
                    waits = list(si.on_wait)
                    cap = 2 if isinstance(inst, mybir.InstEventSemaphore) else 1
                    if len(waits) > cap:
                        extra = waits[: len(waits) - cap]
                        keep = waits[len(waits) - cap:]
                        for k in range(0, len(extra), 2):
                            es = mybir.InstEventSemaphore(
                                name=f"{inst.name}_xw{k}", ins=[], outs=[])
                            es.engine = inst.engine
                            es.sync_info = SyncInfo(
                                on_wait=extra[k:k + 2], on_update=[])
                            nc.register_instruction(es)
                            out.append(es)
                        inst.sync_info = SyncInfo(
                            on_wait=keep, on_update=list(si.on_update))
                        changed = True
                out.append(inst)
            if changed:
                b.instructions = out


def build_program(variant="full", reps=1):
    """One SPMD program; all data-dependence is in the input tensors.

    variant: 'full' | 'nocc' (skip AllReduce, use local z) |
             'noA' (skip attention/copy-gate) | 'mmexp' (matmul+exp only)"""
    nc = bass.Bass("TRN2", target_bir_lowering=False, debug=False,
                   num_devices=NCORE)

    hT = nc.dram_tensor("hT", [H, NTOK], FP16, kind="ExternalInput")
    ctxw = nc.dram_tensor("ctxw", [H, B * CW], FP16, kind="ExternalInput")
    h8 = nc.dram_tensor("h8", [128, KP, 2, NTOK], FP8, kind="ExternalInput")
    w8 = nc.dram_tensor("w8", [128, KP, 2, WCOLS], FP8, kind="ExternalInput")
    bcp = nc.dram_tensor("bcp", [128, 1], FP32, kind="ExternalInput")
    amask = nc.dram_tensor("amask", [1, B * CW], FP16, kind="ExternalInput")
    omask = nc.dram_tensor("omask", [128, TT_N * S], FP32,
                           kind="ExternalInput")
    out = nc.dram_tensor("out", [NTOK, WCOLS], BF16, kind="ExternalOutput")

    z_in = [nc.dram_tensor(f"z_in{g}", [128, GT], FP32) for g in range(NG)]
    z_out = [nc.dram_tensor(f"z_out{g}", [128, GT], FP32,
                            addr_space="Shared") for g in range(NG)]

    with tile.TileContext(nc) as tc:
      for _rep in range(reps):
        with tc.tile_pool(name="pers", bufs=1) as pers:
            bcp_sb = pers.tile([128, 1], FP32, name="bcp_sb", tag="bcp_sb")
            nc.sync.dma_start(bcp_sb[:], bcp[:])
            amask_sb = pers.tile([1, B * CW], FP16, name="amask_sb",
                                 tag="amask_sb")
            nc.sync.dma_start(amask_sb[:], amask[:])
            omask_sb = pers.tile([128, TT_N * S], FP32, name="omask_sb",
                                 tag="omask_sb")
            nc.sync.dma_start(omask_sb[:], omask[:])
            ones_sb = pers.tile([1, 64], FP16, name="ones_sb", tag="ones_sb")
            nc.vector.memset(ones_sb[:], 1.0)

            g_all = pers.tile([128, TT_N], FP32, name="g_all", tag="g_all")
            omcp_all = pers.tile([128, TT_N], FP32, name="omcp_all",
                                 tag="omcp_all")
            l1m_all = pers.tile([128, TT_N], FP32, name="l1m_all",
                                tag="l1m_all")
            zall = pers.tile([128, TT_N], FP32, name="zall", tag="zall")
            zfix = pers.tile([128, TT_N], FP32, name="zfix", tag="zfix")
            zz = pers.tile([128, TT_N], FP32, name="zz", tag="zz")
            acol = pers.tile([128, TT_N], FP32, name="acol", tag="acol")
            sfin = pers.tile([128, TT_N], FP32, name="sfin", tag="sfin")

            pc_t = [pers.tile([128, S], FP32, name=f"pc{t}", tag=f"pc{t}")
                    for t in range(TT_N)]
            zparts = [pers.tile([128, 8], FP32, name=f"zp{t}", tag=f"zp{t}")
                      for t in range(TT_N)]

            h8sb = [pers.tile([128, 2, NTOK], FP8, name=f"h8_{kk}",
                              tag=f"h8_{kk}") for kk in range(KP)]
            for kk in range(KP):
                nc.sync.dma_start(h8sb[kk][:], h8[:, kk:kk + 1, :, :])
            wb = [[pers.tile([128, 2, VN], FP8, name=f"wb{vt}_{kk}",
                             tag=f"wb{vt}_{kk}") for kk in range(KP)]
                  for vt in range(8)]
            for vt in range(8):
                vsl = slice(vt * VN, (vt + 1) * VN)
                for kk in range(KP):
                    nc.sync.dma_start(wb[vt][kk][:], w8[:, kk:kk + 1, :, vsl])

            skip_a = variant in ("noA", "mmexp")
            if skip_a:
                nc.vector.memset(l1m_all[:], 0.0)
                nc.vector.memset(omcp_all[:], 1.0)
                for t in range(TT_N):
                    nc.vector.memset(pc_t[t][:], 0.0)
            # ---------------- Phase A: attention + copy gate (bf16) --------
            if not skip_a:
              with (
                tc.tile_pool(name="hc", bufs=1) as hc,
                tc.tile_pool(name="psA", bufs=2, space="PSUM") as psA,
                tc.tile_pool(name="attw", bufs=3) as attw,
              ):
                  hA, cA = [], []
                  for k in range(KT):
                      ha = hc.tile([128, NTOK], FP16, name=f"hA{k}",
                                   tag=f"hA{k}")
                      nc.sync.dma_start(ha[:], hT[k * 128:(k + 1) * 128, :])
                      hA.append(ha)
                      ca = hc.tile([128, B * CW], FP16, name=f"cA{k}",
                                   tag=f"cA{k}")
                      nc.sync.dma_start(ca[:], ctxw[k * 128:(k + 1) * 128, :])
                      cA.append(ca)

                  for tt in range(TT_N):
                      # scores for the 2 batches of this token tile; col 64
                      # of each half is the copy-gate logit.
                      pat = psA.tile([128, CW], FP32, name="pat", tag="pat")
                      for half in range(2):
                          b = 2 * tt + half
                          rs = slice(64 * half, 64 * half + 64)
                          cs = slice(b * 64, (b + 1) * 64)
                          ws = slice(b * CW, (b + 1) * CW)
                          for k in range(KT):
                              nc.tensor.matmul(pat[rs, :], lhsT=hA[k][:, cs],
                                               rhs=cA[k][:, ws],
                                               start=(k == 0), stop=False)
                          nc.tensor.matmul(pat[rs, :], lhsT=ones_sb[:],
                                           rhs=amask_sb[:, ws],
                                           start=False, stop=True)
                      # copy gate from col 64 (both halves at once)
                      nc.scalar.activation(g_all[:, tt:tt + 1],
                                           pat[:, 64:65], AF.Exp,
                                           bias=bcp_sb[:], scale=1.0)
                      cp_col = attw.tile([128, 1], FP32, name="cp_col",
                                         tag="cp_col")
                      nc.scalar.activation(cp_col[:], pat[:, 64:65],
                                           AF.Sigmoid, bias=bcp_sb[:],
                                           scale=1.0)
                      nc.vector.tensor_scalar(
                          out=omcp_all[:, tt:tt + 1], in0=cp_col[:],
                          scalar1=-1.0, scalar2=1.0, op0=OP.mult, op1=OP.add)
                      nc.scalar.activation(l1m_all[:, tt:tt + 1],
                                           omcp_all[:, tt:tt + 1], AF.Ln,
                                           bias=0.0, scale=1.0)
                      # attention softmax (cols 0:64)
                      negmax = attw.tile([128, 1], FP32, name="negmax",
                                         tag="negmax")
                      nc.vector.tensor_reduce(negmax[:], pat[:, 0:64],
                                              axis=mybir.AxisListType.X,
                                              op=OP.max, negate=True)
                      att_e = attw.tile([128, S], FP32, name="att_e",
                                        tag="att_e")
                      rowsum = attw.tile([128, 1], FP32, name="rowsum",
                                         tag="rowsum")
                      nc.scalar.activation(att_e[:], pat[:, 0:64], AF.Exp,
                                           bias=negmax[:], scale=1.0,
                                           accum_out=rowsum[:])
                      rec = attw.tile([128, 1], FP32, name="rec", tag="rec")
                      nc.vector.reciprocal(rec[:], rowsum[:])
                      pg = attw.tile([128, 1], FP32, name="pg", tag="pg")
                      nc.vector.tensor_tensor(out=pg[:], in0=rec[:],
                                              in1=g_all[:, tt:tt + 1],
                                              op=OP.mult)
                      # pc = attns * cp/(1-cp) * ownership-mask
                      nc.vector.tensor_scalar(out=pc_t[tt][:], in0=att_e[:],
                                              scalar1=pg[:], scalar2=None,
                                              op0=OP.mult)
                      nc.vector.tensor_tensor(
                          out=pc_t[tt][:], in0=pc_t[tt][:],
                          in1=omask_sb[:, tt * S:(tt + 1) * S], op=OP.mult)

            # ---------------- Phase B: fp8 matmul + exp + log-space out ----
            with (
                tc.tile_pool(name="lb", bufs=2 * NG) as lb,
                tc.tile_pool(name="psB", bufs=8, space="PSUM") as psB,
                tc.tile_pool(name="scr", bufs=3) as scr,
                tc.tile_pool(name="outp", bufs=6) as outp,
                tc.tile_pool(name="pzp", bufs=4) as pzp,
                tc.tile_pool(name="ebp", bufs=4) as ebp,
                tc.tile_pool(name="post", bufs=4) as post,
            ):
                for g in range(NG):
                    tts = range(g * GT, (g + 1) * GT)
                    Lt = {}
                    for tt in tts:
                        Lt[tt] = lb.tile([128, WCOLS], BF16, name=f"L{tt}",
                                         tag="L")
                    for vtg in range(2):
                        vts = range(vtg * 4, (vtg + 1) * 4)
                        for tt in tts:
                            ns = slice(tt * 128, (tt + 1) * 128)
                            pss = {}
                            for vt in vts:
                                pss[vt] = psB.tile([128, VN], FP32,
                                                   name="mmp", tag="mmp")
                            for kk in range(KP):
                                for vt in vts:
                                    nc.tensor.matmul(
                                        pss[vt][:],
                                        lhsT=h8sb[kk][:, :, ns],
                                        rhs=wb[vt][kk][:],
                                        start=(kk == 0), stop=(kk == KP - 1),
                                        perf_mode=DR)
                            for vt in vts:
                                vsl = slice(vt * VN, (vt + 1) * VN)
                                sc = scr.tile([128, VN], BF16, name="sc",
                                              tag="sc")
                                nc.scalar.activation(
                                    sc[:], pss[vt][:], AF.Exp,
                                    bias=l1m_all[:, tt:tt + 1], scale=INV,
                                    accum_out=zparts[tt][:, vt:vt + 1])
                                nc.vector.tensor_scalar(
                                    out=Lt[tt][:, vsl], in0=pss[vt][:],
                                    scalar1=INV,
                                    scalar2=l1m_all[:, tt:tt + 1],
                                    op0=OP.mult, op1=OP.add)

                    if variant == "mmexp":
                        continue
                    # local z (+ exact zero-col correction) + allreduce
                    gsl = slice(g * GT, (g + 1) * GT)
                    for tt in tts:
                        nc.vector.tensor_reduce(
                            zall[:, tt:tt + 1], zparts[tt][:],
                            axis=mybir.AxisListType.X, op=OP.add)
                    nc.vector.tensor_scalar(
                        out=zfix[:, gsl], in0=omcp_all[:, gsl],
                        scalar1=-NZERO, scalar2=None, op0=OP.mult)
                    nc.vector.tensor_tensor(out=zall[:, gsl],
                                            in0=zall[:, gsl],
                                            in1=zfix[:, gsl], op=OP.add)
                    if variant == "nocc":
                        nc.vector.tensor_scalar(
                            out=zz[:, gsl], in0=zall[:, gsl],
                            scalar1=float(NCORE), scalar2=None, op0=OP.mult)
                    else:
                        nc.sync.dma_start(z_in[g][:], zall[:, gsl])
                        nc.gpsimd.collective_compute(
                            "AllReduce", OP.add,
                            replica_groups=[list(range(NCORE))],
                            ins=[z_in[g][:]], outs=[z_out[g][:]])
                        nc.sync.dma_start(zz[:, gsl], z_out[g][:])

                    # finalize group: out = L + (C + l1m - ln zz); scatter
                    # blocks get exp/add/ln treatment.
                    for tt in tts:
                        ns = slice(tt * 128, (tt + 1) * 128)
                        lnz = post.tile([128, 1], FP32, name="lnz", tag="lnz")
                        nc.scalar.activation(lnz[:], zz[:, tt:tt + 1], AF.Ln,
                                             bias=0.0,
                                             scale=float(np.exp(-C_CONST)))
                        nc.vector.tensor_scalar(
                            out=acol[:, tt:tt + 1], in0=lnz[:], scalar1=-1.0,
                            scalar2=l1m_all[:, tt:tt + 1], op0=OP.mult,
                            op1=OP.add)
                        nc.scalar.activation(sfin[:, tt:tt + 1],
                                             acol[:, tt:tt + 1], AF.Exp,
                                             bias=0.0, scale=1.0)
                        pcz = pzp.tile([128, S], FP32, name="pcz", tag="pcz")
                        nc.vector.tensor_scalar(
                            out=pcz[:], in0=pc_t[tt][:],
                            scalar1=zz[:, tt:tt + 1], scalar2=None,
                            op0=OP.mult)
                        for vt in range(8):
                            vsl = slice(vt * VN, (vt + 1) * VN)
                            ob = outp.tile([128, VN], BF16, name="ob",
                                           tag="ob")
                            nc.gpsimd.tensor_scalar(
                                out=ob[:], in0=Lt[tt][:, vsl],
                                scalar1=acol[:, tt:tt + 1], scalar2=None,
                                op0=OP.add)
                            for half in range(2):
                                b = 2 * tt + half
                                if b // 7 != vt:
                                    continue
                                rs = slice(64 * half, 64 * half + 64)
                                bc = (b % 7) * 64
                                gc = vt * VN + bc
                                eb = ebp.tile([128, 64], FP32, name="eb",
                                              tag="eb")
                                nc.scalar.activation(eb[rs, :],
                                                     Lt[tt][rs, gc:gc + 64],
                                                     AF.Exp, bias=0.0,
                                                     scale=1.0)
                                nc.vector.tensor_tensor(
                                    out=eb[rs, :], in0=eb[rs, :],
                                    in1=pcz[rs, :], op=OP.add)
                                nc.scalar.activation(
                                    ob[rs, bc:bc + 64], eb[rs, :], AF.Ln,
                                    bias=0.0,
                                    scale=sfin[rs, tt:tt + 1])
                            nc.sync.dma_start(out[ns, vsl], ob[:])

                if variant == "mmexp":
                    nc.sync.dma_start(out[0:128, 0:TT_N], zall[:])

    _split_multi_waits(nc)
    return nc


# ----------------------------------------------------------------------------
# host-side sharding / permutation / assembly
# ----------------------------------------------------------------------------

def _prep_inputs(hidden, context, src, W_gen, b_gen, w_copy, b_copy):
    import ml_dtypes
    assert hidden.shape == (T, B, H) and context.shape == (S, B, H)
    assert W_gen.shape == (V, H) and src.shape == (B, S)
    if not np.all(np.asarray(b_gen) == 0.0):
        raise NotImplementedError("b_gen expected to be all zeros per spec")

    hidden = np.asarray(hidden, np.float32)
    hTf = np.ascontiguousarray(
        hidden.transpose(2, 1, 0).reshape(H, NTOK))          # [H, B*T]
    hT = hTf.astype(np.float16)
    ctxT = np.asarray(context, np.float32).transpose(2, 1, 0)  # [H, B, S]
    ctxw = np.zeros((H, B * CW), np.float16)
    for b in range(B):
        ctxw[:, b * CW:b * CW + S] = ctxT[:, b, :].astype(np.float16)
        ctxw[:, b * CW + S] = np.asarray(w_copy, np.float32).astype(np.float16)
    bcp = np.full((128, 1), float(np.asarray(b_copy).reshape(-1)[0]),
                  np.float32)

    src = np.asarray(src).astype(np.int64)
    amask = np.zeros((1, B * CW), np.float32)
    for b in range(B):
        amask[0, b * CW:b * CW + S] = np.where(src[b] == 0,
                                               np.float32(NEG_BIG), 0.0)
    amask = amask.astype(np.float16)

    # hidden fp8: [128, KP, 2, NTOK], plane (kk, i) = H rows
    # [kk*256 + i*128, +128)
    h8 = np.clip(hTf * HS, -240.0, 240.0).reshape(
        KP, 2, 128, NTOK).transpose(2, 0, 1, 3)
    h8 = np.ascontiguousarray(h8).astype(ml_dtypes.float8_e4m3)

    Wf = np.asarray(W_gen, np.float32)
    per_core = []
    for c in range(NCORE):
        lo, hi = c * VS, (c + 1) * VS
        col_vocab = np.full(WCOLS, -1, np.int64)   # vocab id per column
        placed = {}                                # vocab id -> W-carrying col
        own_pairs = []                             # (b, s, col)
        for b in range(B):
            base = blk0(b)
            for s in range(S):
                v = int(src[b, s])
                if v == 0 or not (lo <= v < hi):
                    continue
                j = base + s
                col_vocab[j] = v
                own_pairs.append((b, s, j))
                if v not in placed:
                    placed[v] = j
        free_cols = np.nonzero(col_vocab < 0)[0]
        remaining = sorted(set(range(lo, hi)) - set(placed.keys()))
        assert len(remaining) + 1 <= len(free_cols), (
            f"core {c}: need {len(remaining)}+pad cols, "
            f"have {len(free_cols)}")
        for idx, v in enumerate(remaining):
            j = int(free_cols[idx])
            col_vocab[j] = v
            placed[v] = j
        padcol = int(free_cols[len(remaining)])
        assert len(placed) == VS

        # W (scaled fp8, permuted); dup + pad cols stay zero
        vids = np.fromiter(placed.keys(), np.int64, len(placed))
        cols = np.fromiter((placed[int(v)] for v in vids), np.int64,
                           len(vids))
        Wcols = np.zeros((H, WCOLS), np.float32)
        Wcols[:, cols] = Wf[vids, :].T
        w8 = np.clip(Wcols * WS, -240.0, 240.0).reshape(
            KP, 2, 128, WCOLS).transpose(2, 0, 1, 3)
        w8 = np.ascontiguousarray(w8).astype(ml_dtypes.float8_e4m3)

        # ownership mask [128, TT_N*S]: row of tile tt is token
        # n = tt*128 + p (batch b = 2*tt + p//64); col group tt, col s
        om = np.zeros((128, TT_N * S), np.float32)
        for (b, s, j) in own_pairs:
            tt, half = b // 2, b % 2
            om[64 * half:64 * half + 64, tt * S + s] = 1.0

        per_core.append(dict(
            in_map={"hT": hT, "ctxw": ctxw, "h8": h8, "w8": w8,
                    "bcp": bcp, "amask": amask, "omask": om},
            col_vocab=col_vocab, placed=placed,
            own_pairs=own_pairs, padcol=padcol,
        ))
    return per_core


def _assemble(per_core, results):
    """results[c]['out'] is [NTOK, WCOLS] bf16 (token n = b*64+t). Returns
    the full [T, B, V] float32 output."""
    big = np.empty((NTOK, V), np.float32)
    for c in range(NCORE):
        o = np.asarray(results[c]["out"], dtype=np.float32)
        meta = per_core[c]
        placed = meta["placed"]
        vids = np.fromiter(placed.keys(), np.int64, len(placed))
        cols = np.fromiter((placed[int(v)] for v in vids), np.int64,
                           len(vids))
        big[:, vids] = o[:, cols]
        # per-batch merges where a batch hit the same vocab at several
        # source positions, or at a non-primary column
        pair_cols = {}
        for (b, s, j) in meta["own_pairs"]:
            v = int(meta["col_vocab"][j])
            pair_cols.setdefault((b, v), []).append(j)
        padcol = meta["padcol"]
        for (b, v), jlist in pair_cols.items():
            prim = placed[v]
            extra = [j for j in jlist if j != prim]
            if not extra:
                continue  # single hit carried by the primary column
            rows = slice(b * T, (b + 1) * T)
            acc = np.exp(o[rows, prim].astype(np.float64))
            base = np.exp(o[rows, padcol].astype(np.float64))
            for j in extra:
                acc += np.exp(o[rows, j].astype(np.float64)) - base
            big[rows, v] = np.log(acc).astype(np.float32)
    return np.ascontiguousarray(
        big.reshape(B, T, V).transpose(1, 0, 2)).astype(np.float32)


_PROGRAM_CACHE = {}


def _get_program():
    if "nc" not in _PROGRAM_CACHE:
        _PROGRAM_CACHE["nc"] = build_program()
    return _PROGRAM_CACHE["nc"]


def kernel(hidden, context, src, W_gen, b_gen, w_copy, b_copy):
    per_core = _prep_inputs(hidden, context, src, W_gen, b_gen, w_copy,
                            b_copy)
    nc = _get_program()
    in_maps = [pc["in_map"] for pc in per_core]
    last_err = None
    for attempt in range(3):
        try:
            res = run_bass_kernel_spmd(nc, in_maps, list(range(NCORE)))
            break
        except Exception as e:  # transient device errors: retry
            last_err = e
            if "UNRECOVERABLE" in str(e) or "UNAVAILABLE" in str(e):
                time.sleep(15)
                continue
            raise
    else:
        raise last_err
    return _assemble(per_core, res.results)
